# revision 29
# baseline (speedup 1.0000x reference)
"""GCN (3-layer GraphConv, norm='both') on 8 Trainium2 NeuronCores — v4.

Self-contained: takes FULL inputs, returns FULL output [N, n_classes].

v4 design (vs v3)
-----------------
v3 moved the segment reduce to the PE array (one-hot sel matmuls) which
unblocked SWDGE descriptor generation; the Pool engine then became the
bottleneck (83% busy) because each dma_gather call costs ~1us fixed on
the Q7 descriptor generator and v3 issued one call per (tile, chunk).

v4 batches gathers across SUPERTILES of ST dst tiles: slots are laid out
per supertile as (chunk, tile) runs, so one gather call covers up to
MAXK groups spanning several tiles' segments for one chunk pad. Calls
per layer drop ~147 -> ~30. sel one-hots are built per supertile in one
DVE op. Inter-tile boundary pads inside a call gather row 0 (valid) with
dstof=-1; trailing pads of a call are skipped via num_idxs.
"""

import math
import os

import numpy as np

P = 128
NCORES = 8
NCH = 3                                        # AllGather chunks per layer
ST = int(os.environ.get("GCN_ST", "1"))        # tiles per supertile
MAXK = int(os.environ.get("GCN_MAXK", "8"))    # max groups per gather call
SCRATCH = int(os.environ.get("GCN_SCRATCH", "16384"))  # SWDGE desc ring bytes
NSWQ = int(os.environ.get("GCN_NSWQ", "4"))    # SWDGE queues (round-robin)
GB_BUFS = int(os.environ.get("GCN_GBBUFS", "3"))
SINGLE_PACKET = os.environ.get("GCN_SP", "1") == "1"


# ----------------------------------------------------------------------------
# Host-side preprocessing
# ----------------------------------------------------------------------------

def _preprocess(features, edge_index, W1, b1, W2, b2, W3, b3):
    n, din = features.shape
    dhid = W2.shape[0]
    ncls = W3.shape[1]
    assert din == P and dhid == P, "kernel assumes 128-wide features"
    assert not (np.any(b1) or np.any(b2)), "nonzero hidden bias unsupported"

    src = np.asarray(edge_index[0], dtype=np.int64)
    dst = np.asarray(edge_index[1], dtype=np.int64)

    deg_out = np.bincount(src, minlength=n).astype(np.float32)
    deg_in = np.bincount(dst, minlength=n).astype(np.float32)
    ns = np.maximum(deg_out, 1.0) ** -0.5
    nd = np.maximum(deg_in, 1.0) ** -0.5

    tpc = math.ceil(n / (P * NCORES))              # tiles per core (49)
    s0 = tpc * P                                   # slots per core
    ntot = NCORES * s0

    # chunk split (in tiles): last chunk smaller so its AllGather (the
    # layer-boundary critical path) fires earlier and lands faster
    if tpc == 49 and NCH == 3:
        chunks = [22, 22, 5]
    else:
        c0 = tpc // NCH
        chunks = [c0] * (NCH - 1) + [tpc - c0 * (NCH - 1)]
    cs = np.concatenate([[0], np.cumsum(chunks)])[:-1]          # start tile
    rows_ch = [chunks[ch] * NCORES * P for ch in range(NCH)]
    assert all(r <= 32767 for r in rows_ch), "chunk rows must fit int16"
    chunk_of_tile = np.searchsorted(cs, np.arange(tpc), side="right") - 1

    # --- node placement: serpentine deal by deg_in desc into (tile, core)
    nbins = tpc * NCORES
    order = np.argsort(-deg_in, kind="stable")
    i = np.arange(n)
    r = i // nbins                                 # slot within bin (= p)
    b = i % nbins
    odd = (r % 2) == 1
    b = np.where(odd, nbins - 1 - b, b)
    j_of_old = np.empty(n, dtype=np.int64)
    c_of_old = np.empty(n, dtype=np.int64)
    p_of_old = np.empty(n, dtype=np.int64)
    j_of_old[order] = b // NCORES
    c_of_old[order] = b % NCORES
    p_of_old[order] = r

    # --- edge mapping
    ce = c_of_old[dst]
    je = j_of_old[dst]
    pe = p_of_old[dst]
    js = j_of_old[src]
    chs = chunk_of_tile[js]
    loc = (c_of_old[src] * np.array(chunks)[chs] * P
           + (js - cs[chs]) * P + p_of_old[src])
    assert loc.max() < 32768

    # sort edges by (core, tile, src-chunk, src-loc)
    okey = np.lexsort((loc, chs, je, ce))
    ce, je, pe, chs, loc = ce[okey], je[okey], pe[okey], chs[okey], loc[okey]

    # dedup: one gather slot per unique (core, tile, chunk, src); a slot
    # feeding multiple dsts (or a multi-edge) gets a multi-hot sel column
    flatkey = (ce * tpc + je) * NCH + chs
    new_slot = np.ones(len(flatkey), dtype=bool)
    new_slot[1:] = (flatkey[1:] != flatkey[:-1]) | (loc[1:] != loc[:-1])
    u_key = flatkey[new_slot]
    cnt = np.bincount(u_key, minlength=NCORES * tpc * NCH).reshape(
        NCORES, tpc, NCH)
    cap = cnt.max(axis=0)                          # [tpc, NCH]
    gseg = -(-cap // P)                            # groups per (tile, chunk)

    # edge-level ranges + per-edge unique-slot position within its segment
    ecnt = np.bincount(flatkey, minlength=NCORES * tpc * NCH)
    edge_start = np.zeros(NCORES * tpc * NCH + 1, dtype=np.int64)
    np.cumsum(ecnt, out=edge_start[1:])
    slot_ord = np.cumsum(new_slot) - 1
    u_start = np.zeros(NCORES * tpc * NCH + 1, dtype=np.int64)
    np.cumsum(cnt.reshape(-1), out=u_start[1:])
    pos_e = slot_ord - u_start[flatkey]
    u_loc = loc[new_slot]

    # --- supertiles: groups laid out per st as (chunk, tile) runs
    sts = [list(range(t0, min(t0 + ST, tpc))) for t0 in range(0, tpc, ST)]
    st_of_tile = {}
    for si, tl in enumerate(sts):
        for j in tl:
            st_of_tile[j] = si

    g_of = {}                  # (j, ch) -> (group offset within st, ngroups)
    st_groups = []             # groups per supertile
    og_st = []                 # global dstof col offset per supertile
    call_list = []             # (si, ch, o_g, coff, num_idxs)
    # per-core fill info per (si, ch): list of (j, o_g)
    run_of = {}
    og = 0
    coff = 0
    for si, tl in enumerate(sts):
        og_st.append(og)
        g = 0
        for ch in range(NCH):
            run = []
            run_g0 = g
            for j in tl:
                g_of[(j, ch)] = (g, int(gseg[j, ch]))
                run.append((j, g))
                g += int(gseg[j, ch])
            run_of[(si, ch)] = run
            # per-group real-slot counts for this run (pad tail only in each
            # tile's final group)
            greal = []
            for j in tl:
                gs = int(gseg[j, ch])
                for gi in range(gs):
                    if gi == gs - 1:
                        greal.append(int(cap[j, ch]) - (gs - 1) * P)
                    else:
                        greal.append(P)
            # greedy pack up to MAXK groups per call; trailing pad of the
            # call's last group is skipped via num_idxs
            g0 = 0
            while g0 < len(greal):
                k = min(MAXK, len(greal) - g0)
                num = (k - 1) * P + greal[g0 + k - 1]
                cols = -(-num // 16)
                call_list.append((si, ch, run_g0 + g0, coff, num))
                coff += cols
                g0 += k
        st_groups.append(g)
        og += g
    Gtot = og
    Gsup = max(st_groups)
    idx_cols = max(coff, 8)

    # --- per-core idx buffers (unique slots) + multi-hot sel counts
    idx_all = np.zeros((NCORES, P, idx_cols), dtype=np.int16)
    base_g = np.zeros((tpc, NCH), dtype=np.int64)   # global group col base
    for j in range(tpc):
        for ch in range(NCH):
            base_g[j, ch] = og_st[st_of_tile[j]] + g_of[(j, ch)][0]
    gcol_e = base_g[je, chs] + pos_e // P
    pslot_e = pos_e % P
    selcnt = np.zeros((NCORES, P, Gtot, P), dtype=np.uint8)
    np.add.at(selcnt, (ce, pslot_e, gcol_e, pe), 1)
    for c in range(NCORES):
        flat_idx = {}
        for si, tl in enumerate(sts):
            for ch in range(NCH):
                parts_i = []
                for j in tl:
                    k = (c * tpc + j) * NCH + ch
                    u0, u1 = u_start[k], u_start[k + 1]
                    gs = int(gseg[j, ch])
                    li = np.zeros(gs * P, dtype=np.int16)
                    li[:u1 - u0] = u_loc[u0:u1]
                    parts_i.append(li)
                fi = np.concatenate(parts_i) if parts_i else np.zeros(
                    0, np.int16)
                flat_idx[(si, ch)] = fi
        for (si, ch, o_g, cof, num) in call_list:
            o_run = run_of[(si, ch)][0][1]
            fi = flat_idx[(si, ch)]
            s_lo = (o_g - o_run) * P
            flat = fi[s_lo:s_lo + num]
            cols = -(-num // 16)
            fpad = np.zeros(cols * 16, dtype=np.int16)
            fpad[:num] = flat
            w16 = fpad.reshape(cols, 16).T
            idx_all[c, :, cof:cof + cols] = np.tile(w16, (8, 1))

    # --- per-core dense inputs
    new_flat = (c_of_old * tpc + j_of_old) * P + p_of_old
    old_of_new = np.full(ntot, -1, dtype=np.int64)
    old_of_new[new_flat] = np.arange(n)

    feat_new = np.zeros((ntot, din), dtype=np.float32)
    feat_new[new_flat] = np.asarray(features, dtype=np.float32)
    ns_new = np.zeros(ntot, dtype=np.float32)
    ns_new[new_flat] = ns
    nd_new = np.zeros(ntot, dtype=np.float32)
    nd_new[new_flat] = nd

    def per_core_scale(vec):
        return [np.ascontiguousarray(
            vec[c * s0:(c + 1) * s0].reshape(tpc, P).T) for c in range(NCORES)]

    xct = [np.ascontiguousarray(
        feat_new[c * s0:(c + 1) * s0].T.astype(np.float16))
        for c in range(NCORES)]
    s0_scale = per_core_scale(ns_new)
    s12_scale = per_core_scale(ns_new * nd_new)
    s3_scale = per_core_scale(nd_new)

    w3p = np.zeros((dhid, P), dtype=np.float16)
    w3p[:, :ncls] = np.asarray(W3, dtype=np.float32).astype(np.float16)

    # host-built fp8 one-hot sel masks: sel[p, g*128+d] = (dstof[p,g] == d)
    import ml_dtypes
    f8np = ml_dtypes.float8_e4m3
    dcol = np.arange(P, dtype=np.float16)
    selt_all = [
        (dstof_all[c][:, :, None] == dcol[None, None, :]).astype(f8np).reshape(
            P, Gtot * P)
        for c in range(NCORES)
    ]

    meta = dict(
        n=n, din=din, dhid=dhid, ncls=ncls, tpc=tpc, s0=s0, ntot=ntot,
        chunks=chunks, cs=cs.tolist(), rows_ch=rows_ch,
        sts=sts, g_of=g_of, og_st=og_st, st_groups=st_groups,
        call_list=call_list, idx_cols=idx_cols,
        Gsup=Gsup, Gtot=Gtot,
        slots=int(cap.sum()),
        old_of_new=old_of_new, b3=np.asarray(b3, np.float32),
    )
    in_maps = []
    for c in range(NCORES):
        in_maps.append({
            "xct": xct[c],
            "w1": np.asarray(W1, np.float32).astype(np.float16),
            "w2": np.asarray(W2, np.float32).astype(np.float16),
            "w3p": w3p,
            "sc0": s0_scale[c],
            "sc12": s12_scale[c],
            "sc3": s3_scale[c],
            "idx": np.ascontiguousarray(idx_all[c]),
            "selt": np.ascontiguousarray(selt_all[c]),
        })
    return meta, in_maps


# ----------------------------------------------------------------------------
# Device program
# ----------------------------------------------------------------------------

def _build_program(meta, enable_asserts=False):
    import concourse.bacc as bacc
    import concourse.mybir as mybir
    import concourse.tile as tile
    from concourse.masks import make_identity

    f32 = mybir.dt.float32
    f16 = mybir.dt.float16
    f8 = mybir.dt.float8e4
    i16 = mybir.dt.int16
    Alu = mybir.AluOpType
    Act = mybir.ActivationFunctionType

    tpc, s0 = meta["tpc"], meta["s0"]
    dhid = meta["dhid"]
    chunks, cs, rows_ch = meta["chunks"], meta["cs"], meta["rows_ch"]
    sts, g_of, og_st = meta["sts"], meta["g_of"], meta["og_st"]
    st_groups = meta["st_groups"]
    call_list = meta["call_list"]
    Gsup, Gtot, idx_cols = meta["Gsup"], meta["Gtot"], meta["idx_cols"]

    calls_by_st = [[] for _ in sts]
    for (si, ch, o_g, coff, num) in call_list:
        calls_by_st[si].append((ch, o_g, coff, num))

    fire_pos = {cs[ch] + chunks[ch] - 1: ch for ch in range(NCH)}

    nc = bacc.Bacc(
        "TRN2", target_bir_lowering=False, debug=False,
        enable_asserts=enable_asserts, num_devices=NCORES,
        num_swdge_queues=NSWQ, dynamic_dma_scratch_size=SCRATCH,
    )
    qload = [0.0] * NSWQ

    def next_queue(rows):
        q = min(range(NSWQ), key=lambda i: qload[i])
        qload[q] += rows + 3000.0
        return q

    xct = nc.dram_tensor("xct", [P, s0], f16, kind="ExternalInput")
    w1 = nc.dram_tensor("w1", [P, dhid], f16, kind="ExternalInput")
    w2 = nc.dram_tensor("w2", [dhid, dhid], f16, kind="ExternalInput")
    w3p = nc.dram_tensor("w3p", [dhid, P], f16, kind="ExternalInput")
    sc0 = nc.dram_tensor("sc0", [P, tpc], f32, kind="ExternalInput")
    sc12 = nc.dram_tensor("sc12", [P, tpc], f32, kind="ExternalInput")
    sc3 = nc.dram_tensor("sc3", [P, tpc], f32, kind="ExternalInput")
    idx = nc.dram_tensor("idx", [P, idx_cols], i16, kind="ExternalInput")
    selt = nc.dram_tensor("selt", [P, Gtot * P], f8, kind="ExternalInput")
    outp = nc.dram_tensor("outp", [s0, P], f32, kind="ExternalOutput")

    rg = [list(range(NCORES))]

    with tile.TileContext(nc) as tc:
        with (
            tc.tile_pool(name="constp", bufs=1) as constp,
            tc.tile_pool(name="gatherp", bufs=GB_BUFS) as gatherp,
            tc.tile_pool(name="workp", bufs=4) as workp,
            tc.tile_pool(name="psumap", bufs=2, space="PSUM") as psumap,
            tc.tile_pool(name="psumtp", bufs=2, space="PSUM") as psumtp,
            tc.tile_pool(name="psumzp", bufs=2, space="PSUM") as psumzp,
            tc.tile_pool(name="dramp", bufs=1, space="DRAM") as dramp,
        ):
            z1 = dramp.tile([s0, dhid], f16)
            z2 = dramp.tile([s0, dhid], f16)
            z3 = dramp.tile([s0, P], f16)
            # per-(layer, chunk) AllGather landing pads; gathers read these
            # directly with chunk-local indices
            pads = []
            for li, d in ((0, dhid), (1, dhid), (2, P)):
                pads.append([dramp.tile(
                    [rows_ch[ch], d], f16, addr_space="Shared",
                    name=f"pad{li}_{ch}") for ch in range(NCH)])

            xct_sb = constp.tile([P, s0], f16)
            nc.sync.dma_start(out=xct_sb[:], in_=xct[:, :])
            w1_sb = constp.tile([P, dhid], f16)
            nc.sync.dma_start(out=w1_sb[:], in_=w1[:, :])
            w2_sb = constp.tile([P, dhid], f16)
            nc.sync.dma_start(out=w2_sb[:], in_=w2[:, :])
            w3_sb = constp.tile([P, P], f16)
            nc.sync.dma_start(out=w3_sb[:], in_=w3p[:, :])
            sc0_sb = constp.tile([P, tpc], f32)
            nc.sync.dma_start(out=sc0_sb[:], in_=sc0[:, :])
            sc12_sb = constp.tile([P, tpc], f32)
            nc.sync.dma_start(out=sc12_sb[:], in_=sc12[:, :])
            sc3_sb = constp.tile([P, tpc], f32)
            nc.sync.dma_start(out=sc3_sb[:], in_=sc3[:, :])
            idx_sb = constp.tile([P, idx_cols], i16)
            nc.sync.dma_start(out=idx_sb[:], in_=idx[:, :])
            sel_sb = constp.tile([P, Gtot, P], f8)
            ident = constp.tile([P, P], f16)
            make_identity(nc, ident[:])

            # pre-zero the gather bufs once so unwritten tail slots read 0.0
            for _ in range(GB_BUFS):
                gz = gatherp.tile([P, Gsup, P], f16, tag="gb")
                nc.vector.memset(gz[:], 0.0)

            def chunk_collective(zbuf, li, ch):
                zlo, zhi = cs[ch] * P, (cs[ch] + chunks[ch]) * P
                nc.gpsimd.collective_compute(
                    "AllGather", Alu.bypass, replica_groups=rg,
                    ins=[zbuf[zlo:zhi, :].opt()],
                    outs=[pads[li][ch][:, :].opt()],
                )

            # ---- phase 0: z1 = ns * (X @ W1) -> chunked AllGather -> pads[0]
            # the 14.5MB sel table load is split into per-tile pieces
            # interleaved with the z1 writes so it doesn't head-block the
            # Sync queue (z writes feed the chunk collectives)
            sel_cut = [0] * (tpc + 1)
            for j in range(tpc):
                sel_cut[j + 1] = Gtot * (j + 1) // tpc
            for j in range(tpc):
                zp = psumzp.tile([P, dhid], f32, tag="zp")
                nc.tensor.matmul(
                    out=zp[:], lhsT=xct_sb[:, j * P:(j + 1) * P], rhs=w1_sb[:],
                    start=True, stop=True,
                )
                zt = workp.tile([P, dhid], f16, tag="zt")
                nc.scalar.activation(
                    out=zt[:], in_=zp[:], func=Act.Copy,
                    scale=sc0_sb[:, j:j + 1],
                )
                nc.sync.dma_start(out=z1[j * P:(j + 1) * P, :], in_=zt[:])
                if j in fire_pos:
                    chunk_collective(z1, 0, fire_pos[j])
                g0, g1 = sel_cut[j], sel_cut[j + 1]
                if g1 > g0:
                    nc.sync.dma_start(
                        out=sel_sb[:, g0:g1, :],
                        in_=selt[:, g0 * P:g1 * P],
                    )

            def spmm_layer(li, d_el, scale_sb, wnext_sb, zout, li_out):
                for si, tl in enumerate(sts):
                    gb = gatherp.tile([P, Gsup, P], f16, tag="gb")
                    for (ch, o_g, coff, num) in calls_by_st[si]:
                        K = -(-num // P)
                        nc.gpsimd.dma_gather(
                            out_ap=gb[:, o_g:o_g + K, :],
                            in_ap=pads[li][ch][:, :],
                            idxs_ap=idx_sb[:, coff:coff + (-(-num // 16))],
                            num_idxs=num, num_idxs_reg=num,
                            elem_size=d_el, single_packet=SINGLE_PACKET,
                            queue_num=next_queue(num),
                        )
                    og = og_st[si]
                    for j in tl:
                        glist = []
                        for ch in range(NCH):
                            o, ng = g_of[(j, ch)]
                            glist.extend(range(o, o + ng))
                        acc = psumap.tile([P, P], f32, tag="acc")
                        for gi, g in enumerate(glist):
                            nc.tensor.matmul(
                                out=acc[:], lhsT=sel_sb[:, og + g, :],
                                rhs=gb[:, g, :],
                                start=(gi == 0), stop=(gi == len(glist) - 1),
                            )
                        if wnext_sb is None:
                            ot = workp.tile([P, P], f32, tag="ot")
                            nc.scalar.activation(
                                out=ot[:], in_=acc[:], func=Act.Copy,
                                scale=scale_sb[:, j:j + 1],
                            )
                            nc.sync.dma_start(
                                out=outp[j * P:(j + 1) * P, :], in_=ot[:])
                        else:
                            ht = workp.tile([P, P], f16, tag="ht")
                            nc.scalar.activation(
                                out=ht[:], in_=acc[:], func=Act.Relu,
                                scale=scale_sb[:, j:j + 1],
                            )
                            tp = psumtp.tile([P, P], f16, tag="tp")
                            nc.tensor.transpose(out=tp[:], in_=ht[:],
                                                identity=ident[:])
                            htT = workp.tile([P, P], f16, tag="htT")
                            nc.scalar.activation(out=htT[:], in_=tp[:],
                                                 func=Act.Copy)
                            zp = psumzp.tile([P, P], f32, tag="zp2")
                            nc.tensor.matmul(
                                out=zp[:], lhsT=htT[:], rhs=wnext_sb[:],
                                start=True, stop=True,
                            )
                            zt = workp.tile([P, P], f16, tag="zt2")
                            nc.scalar.activation(out=zt[:], in_=zp[:],
                                                 func=Act.Copy)
                            nc.sync.dma_start(
                                out=zout[j * P:(j + 1) * P, :], in_=zt[:])
                            if j in fire_pos:
                                chunk_collective(zout, li_out, fire_pos[j])

            spmm_layer(0, dhid, sc12_sb, w2_sb, z2, 1)
            spmm_layer(1, dhid, sc12_sb, w3_sb, z3, 2)
            spmm_layer(2, P, sc3_sb, None, None, None)

    nc.compile()
    return nc


# ----------------------------------------------------------------------------
# Entry point
# ----------------------------------------------------------------------------

_CACHE = {}


def _graph_key(edge_index, shapes):
    e = np.asarray(edge_index)
    return (e.shape, hash(e.tobytes()), shapes)


def run(inputs, trace=False, trace_cores=None):
    features = np.asarray(inputs["features"], dtype=np.float32)
    edge_index = np.asarray(inputs["edge_index"])
    W1, b1 = np.asarray(inputs["W1"]), np.asarray(inputs["b1"])
    W2, b2 = np.asarray(inputs["W2"]), np.asarray(inputs["b2"])
    W3, b3 = np.asarray(inputs["W3"]), np.asarray(inputs["b3"])

    meta, in_maps = _preprocess(features, edge_index, W1, b1, W2, b2, W3, b3)
    key = _graph_key(edge_index, (features.shape,))
    if key not in _CACHE:
        _CACHE[key] = _build_program(meta)
    nc = _CACHE[key]

    import concourse.bass_utils as bass_utils

    res = bass_utils.run_bass_kernel_spmd(
        nc, in_maps, core_ids=list(range(NCORES)),
        trace=trace, trace_cores=trace_cores,
    )
    return _assemble(meta, [r["outp"] for r in res.results]), res


def kernel(**inputs):
    return run(inputs)[0]


def _assemble(meta, outs):
    n, ncls, s0 = meta["n"], meta["ncls"], meta["s0"]
    old_of_new = meta["old_of_new"]
    b3 = meta["b3"]
    result = np.empty((n, ncls), dtype=np.float32)
    for c in range(NCORES):
        ids = old_of_new[c * s0:(c + 1) * s0]
        m = ids >= 0
        arr = np.asarray(outs[c])                  # [s0, P]
        result[ids[m]] = arr[m][:, :ncls]
    result += b3[None, :]
    return result


# revision 31
# speedup vs baseline: 1.0815x; 1.0815x over previous
"""GCN (3-layer GraphConv, norm='both') on 8 Trainium2 NeuronCores — v4.

Self-contained: takes FULL inputs, returns FULL output [N, n_classes].

v4 design (vs v3)
-----------------
v3 moved the segment reduce to the PE array (one-hot sel matmuls) which
unblocked SWDGE descriptor generation; the Pool engine then became the
bottleneck (83% busy) because each dma_gather call costs ~1us fixed on
the Q7 descriptor generator and v3 issued one call per (tile, chunk).

v4 batches gathers across SUPERTILES of ST dst tiles: slots are laid out
per supertile as (chunk, tile) runs, so one gather call covers up to
MAXK groups spanning several tiles' segments for one chunk pad. Calls
per layer drop ~147 -> ~30. sel one-hots are built per supertile in one
DVE op. Inter-tile boundary pads inside a call gather row 0 (valid) with
dstof=-1; trailing pads of a call are skipped via num_idxs.
"""

import math
import os

import numpy as np

P = 128
NCORES = 8
NCH = 3                                        # AllGather chunks per layer
ST = int(os.environ.get("GCN_ST", "1"))        # tiles per supertile
MAXK = int(os.environ.get("GCN_MAXK", "8"))    # max groups per gather call
SCRATCH = int(os.environ.get("GCN_SCRATCH", "16384"))  # SWDGE desc ring bytes
NSWQ = int(os.environ.get("GCN_NSWQ", "4"))    # SWDGE queues (round-robin)
GB_BUFS = int(os.environ.get("GCN_GBBUFS", "3"))
SINGLE_PACKET = os.environ.get("GCN_SP", "1") == "1"


# ----------------------------------------------------------------------------
# Host-side preprocessing
# ----------------------------------------------------------------------------

def _preprocess(features, edge_index, W1, b1, W2, b2, W3, b3):
    n, din = features.shape
    dhid = W2.shape[0]
    ncls = W3.shape[1]
    assert din == P and dhid == P, "kernel assumes 128-wide features"
    assert not (np.any(b1) or np.any(b2)), "nonzero hidden bias unsupported"

    src = np.asarray(edge_index[0], dtype=np.int64)
    dst = np.asarray(edge_index[1], dtype=np.int64)

    deg_out = np.bincount(src, minlength=n).astype(np.float32)
    deg_in = np.bincount(dst, minlength=n).astype(np.float32)
    ns = np.maximum(deg_out, 1.0) ** -0.5
    nd = np.maximum(deg_in, 1.0) ** -0.5

    tpc = math.ceil(n / (P * NCORES))              # tiles per core (49)
    s0 = tpc * P                                   # slots per core
    ntot = NCORES * s0

    # chunk split (in tiles): last chunk smaller so its AllGather (the
    # layer-boundary critical path) fires earlier and lands faster
    if tpc == 49 and NCH == 3:
        chunks = [20, 20, 9]
    else:
        c0 = tpc // NCH
        chunks = [c0] * (NCH - 1) + [tpc - c0 * (NCH - 1)]
    cs = np.concatenate([[0], np.cumsum(chunks)])[:-1]          # start tile
    rows_ch = [chunks[ch] * NCORES * P for ch in range(NCH)]
    assert all(r <= 32767 for r in rows_ch), "chunk rows must fit int16"
    chunk_of_tile = np.searchsorted(cs, np.arange(tpc), side="right") - 1

    # --- node placement: serpentine deal by deg_in desc into (tile, core)
    nbins = tpc * NCORES
    order = np.argsort(-deg_in, kind="stable")
    i = np.arange(n)
    r = i // nbins                                 # slot within bin (= p)
    b = i % nbins
    odd = (r % 2) == 1
    b = np.where(odd, nbins - 1 - b, b)
    j_of_old = np.empty(n, dtype=np.int64)
    c_of_old = np.empty(n, dtype=np.int64)
    p_of_old = np.empty(n, dtype=np.int64)
    j_of_old[order] = b // NCORES
    c_of_old[order] = b % NCORES
    p_of_old[order] = r

    # --- edge mapping
    ce = c_of_old[dst]
    je = j_of_old[dst]
    pe = p_of_old[dst]
    js = j_of_old[src]
    chs = chunk_of_tile[js]
    loc = (c_of_old[src] * np.array(chunks)[chs] * P
           + (js - cs[chs]) * P + p_of_old[src])
    assert loc.max() < 32768

    # sort edges by (core, tile, src-chunk, src-loc)
    okey = np.lexsort((loc, chs, je, ce))
    ce, je, pe, chs, loc = ce[okey], je[okey], pe[okey], chs[okey], loc[okey]

    # dedup: one gather slot per unique (core, tile, chunk, src); a slot
    # feeding multiple dsts (or a multi-edge) gets a multi-hot sel column
    flatkey = (ce * tpc + je) * NCH + chs
    new_slot = np.ones(len(flatkey), dtype=bool)
    new_slot[1:] = (flatkey[1:] != flatkey[:-1]) | (loc[1:] != loc[:-1])
    u_key = flatkey[new_slot]
    cnt = np.bincount(u_key, minlength=NCORES * tpc * NCH).reshape(
        NCORES, tpc, NCH)
    cap = cnt.max(axis=0)                          # [tpc, NCH]
    gseg = -(-cap // P)                            # groups per (tile, chunk)

    # edge-level ranges + per-edge unique-slot position within its segment
    ecnt = np.bincount(flatkey, minlength=NCORES * tpc * NCH)
    edge_start = np.zeros(NCORES * tpc * NCH + 1, dtype=np.int64)
    np.cumsum(ecnt, out=edge_start[1:])
    slot_ord = np.cumsum(new_slot) - 1
    u_start = np.zeros(NCORES * tpc * NCH + 1, dtype=np.int64)
    np.cumsum(cnt.reshape(-1), out=u_start[1:])
    pos_e = slot_ord - u_start[flatkey]
    u_loc = loc[new_slot]

    # --- supertiles: groups laid out per st as (chunk, tile) runs
    sts = [list(range(t0, min(t0 + ST, tpc))) for t0 in range(0, tpc, ST)]
    st_of_tile = {}
    for si, tl in enumerate(sts):
        for j in tl:
            st_of_tile[j] = si

    g_of = {}                  # (j, ch) -> (group offset within st, ngroups)
    st_groups = []             # groups per supertile
    og_st = []                 # global dstof col offset per supertile
    call_list = []             # (si, ch, o_g, coff, num_idxs)
    # per-core fill info per (si, ch): list of (j, o_g)
    run_of = {}
    og = 0
    coff = 0
    for si, tl in enumerate(sts):
        og_st.append(og)
        g = 0
        for ch in range(NCH):
            run = []
            run_g0 = g
            for j in tl:
                g_of[(j, ch)] = (g, int(gseg[j, ch]))
                run.append((j, g))
                g += int(gseg[j, ch])
            run_of[(si, ch)] = run
            # per-group real-slot counts for this run (pad tail only in each
            # tile's final group)
            greal = []
            for j in tl:
                gs = int(gseg[j, ch])
                for gi in range(gs):
                    if gi == gs - 1:
                        greal.append(int(cap[j, ch]) - (gs - 1) * P)
                    else:
                        greal.append(P)
            # greedy pack up to MAXK groups per call; trailing pad of the
            # call's last group is skipped via num_idxs
            g0 = 0
            while g0 < len(greal):
                k = min(MAXK, len(greal) - g0)
                num = (k - 1) * P + greal[g0 + k - 1]
                cols = -(-num // 16)
                call_list.append((si, ch, run_g0 + g0, coff, num))
                coff += cols
                g0 += k
        st_groups.append(g)
        og += g
    Gtot = og
    Gsup = max(st_groups)
    idx_cols = max(coff, 8)

    # --- per-core idx buffers (unique slots) + multi-hot sel counts
    idx_all = np.zeros((NCORES, P, idx_cols), dtype=np.int16)
    base_g = np.zeros((tpc, NCH), dtype=np.int64)   # global group col base
    for j in range(tpc):
        for ch in range(NCH):
            base_g[j, ch] = og_st[st_of_tile[j]] + g_of[(j, ch)][0]
    gcol_e = base_g[je, chs] + pos_e // P
    pslot_e = pos_e % P
    selcnt = np.zeros((NCORES, P, Gtot, P), dtype=np.uint8)
    np.add.at(selcnt, (ce, pslot_e, gcol_e, pe), 1)
    for c in range(NCORES):
        flat_idx = {}
        for si, tl in enumerate(sts):
            for ch in range(NCH):
                parts_i = []
                for j in tl:
                    k = (c * tpc + j) * NCH + ch
                    u0, u1 = u_start[k], u_start[k + 1]
                    gs = int(gseg[j, ch])
                    li = np.zeros(gs * P, dtype=np.int16)
                    li[:u1 - u0] = u_loc[u0:u1]
                    parts_i.append(li)
                fi = np.concatenate(parts_i) if parts_i else np.zeros(
                    0, np.int16)
                flat_idx[(si, ch)] = fi
        for (si, ch, o_g, cof, num) in call_list:
            o_run = run_of[(si, ch)][0][1]
            fi = flat_idx[(si, ch)]
            s_lo = (o_g - o_run) * P
            flat = fi[s_lo:s_lo + num]
            cols = -(-num // 16)
            fpad = np.zeros(cols * 16, dtype=np.int16)
            fpad[:num] = flat
            w16 = fpad.reshape(cols, 16).T
            idx_all[c, :, cof:cof + cols] = np.tile(w16, (8, 1))

    # --- per-core dense inputs
    new_flat = (c_of_old * tpc + j_of_old) * P + p_of_old
    old_of_new = np.full(ntot, -1, dtype=np.int64)
    old_of_new[new_flat] = np.arange(n)

    feat_new = np.zeros((ntot, din), dtype=np.float32)
    feat_new[new_flat] = np.asarray(features, dtype=np.float32)
    ns_new = np.zeros(ntot, dtype=np.float32)
    ns_new[new_flat] = ns
    nd_new = np.zeros(ntot, dtype=np.float32)
    nd_new[new_flat] = nd

    def per_core_scale(vec):
        return [np.ascontiguousarray(
            vec[c * s0:(c + 1) * s0].reshape(tpc, P).T) for c in range(NCORES)]

    xct = [np.ascontiguousarray(
        feat_new[c * s0:(c + 1) * s0].T.astype(np.float16))
        for c in range(NCORES)]
    s0_scale = per_core_scale(ns_new)
    s12_scale = per_core_scale(ns_new * nd_new)
    s3_scale = per_core_scale(nd_new)

    w3p = np.zeros((dhid, P), dtype=np.float16)
    w3p[:, :ncls] = np.asarray(W3, dtype=np.float32).astype(np.float16)

    # host-built fp8 one-hot sel masks: sel[p, g*128+d] = (dstof[p,g] == d)
    import ml_dtypes
    f8np = ml_dtypes.float8_e4m3
    selt_all = [
        selcnt[c].astype(f8np).reshape(P, Gtot * P) for c in range(NCORES)
    ]

    meta = dict(
        n=n, din=din, dhid=dhid, ncls=ncls, tpc=tpc, s0=s0, ntot=ntot,
        chunks=chunks, cs=cs.tolist(), rows_ch=rows_ch,
        sts=sts, g_of=g_of, og_st=og_st, st_groups=st_groups,
        call_list=call_list, idx_cols=idx_cols,
        Gsup=Gsup, Gtot=Gtot,
        slots=int(cap.sum()),
        old_of_new=old_of_new, b3=np.asarray(b3, np.float32),
    )
    in_maps = []
    for c in range(NCORES):
        in_maps.append({
            "xct": xct[c],
            "w1": np.asarray(W1, np.float32).astype(np.float16),
            "w2": np.asarray(W2, np.float32).astype(np.float16),
            "w3p": w3p,
            "sc0": s0_scale[c],
            "sc12": s12_scale[c],
            "sc3": s3_scale[c],
            "idx": np.ascontiguousarray(idx_all[c]),
            "selt": np.ascontiguousarray(selt_all[c]),
        })
    return meta, in_maps


# ----------------------------------------------------------------------------
# Device program
# ----------------------------------------------------------------------------

def _build_program(meta, enable_asserts=False):
    import concourse.bacc as bacc
    import concourse.mybir as mybir
    import concourse.tile as tile
    from concourse.masks import make_identity

    f32 = mybir.dt.float32
    f16 = mybir.dt.float16
    f8 = mybir.dt.float8e4
    i16 = mybir.dt.int16
    Alu = mybir.AluOpType
    Act = mybir.ActivationFunctionType

    tpc, s0 = meta["tpc"], meta["s0"]
    dhid = meta["dhid"]
    chunks, cs, rows_ch = meta["chunks"], meta["cs"], meta["rows_ch"]
    sts, g_of, og_st = meta["sts"], meta["g_of"], meta["og_st"]
    st_groups = meta["st_groups"]
    call_list = meta["call_list"]
    Gsup, Gtot, idx_cols = meta["Gsup"], meta["Gtot"], meta["idx_cols"]

    calls_by_st = [[] for _ in sts]
    for (si, ch, o_g, coff, num) in call_list:
        calls_by_st[si].append((ch, o_g, coff, num))

    fire_pos = {cs[ch] + chunks[ch] - 1: ch for ch in range(NCH)}

    nc = bacc.Bacc(
        "TRN2", target_bir_lowering=False, debug=False,
        enable_asserts=enable_asserts, num_devices=NCORES,
        num_swdge_queues=NSWQ, dynamic_dma_scratch_size=SCRATCH,
    )
    qload = [0.0] * NSWQ

    def next_queue(rows):
        q = min(range(NSWQ), key=lambda i: qload[i])
        qload[q] += rows + 3000.0
        return q

    xct = nc.dram_tensor("xct", [P, s0], f16, kind="ExternalInput")
    w1 = nc.dram_tensor("w1", [P, dhid], f16, kind="ExternalInput")
    w2 = nc.dram_tensor("w2", [dhid, dhid], f16, kind="ExternalInput")
    w3p = nc.dram_tensor("w3p", [dhid, P], f16, kind="ExternalInput")
    sc0 = nc.dram_tensor("sc0", [P, tpc], f32, kind="ExternalInput")
    sc12 = nc.dram_tensor("sc12", [P, tpc], f32, kind="ExternalInput")
    sc3 = nc.dram_tensor("sc3", [P, tpc], f32, kind="ExternalInput")
    idx = nc.dram_tensor("idx", [P, idx_cols], i16, kind="ExternalInput")
    selt = nc.dram_tensor("selt", [P, Gtot * P], f8, kind="ExternalInput")
    outp = nc.dram_tensor("outp", [s0, P], f32, kind="ExternalOutput")

    rg = [list(range(NCORES))]

    with tile.TileContext(nc) as tc:
        with (
            tc.tile_pool(name="constp", bufs=1) as constp,
            tc.tile_pool(name="gatherp", bufs=GB_BUFS) as gatherp,
            tc.tile_pool(name="workp", bufs=4) as workp,
            tc.tile_pool(name="psumap", bufs=2, space="PSUM") as psumap,
            tc.tile_pool(name="psumtp", bufs=2, space="PSUM") as psumtp,
            tc.tile_pool(name="psumzp", bufs=2, space="PSUM") as psumzp,
            tc.tile_pool(name="dramp", bufs=1, space="DRAM") as dramp,
        ):
            z1 = dramp.tile([s0, dhid], f16)
            z2 = dramp.tile([s0, dhid], f16)
            z3 = dramp.tile([s0, P], f16)
            # per-(layer, chunk) AllGather landing pads; gathers read these
            # directly with chunk-local indices
            pads = []
            for li, d in ((0, dhid), (1, dhid), (2, P)):
                pads.append([dramp.tile(
                    [rows_ch[ch], d], f16, addr_space="Shared",
                    name=f"pad{li}_{ch}") for ch in range(NCH)])

            xct_sb = constp.tile([P, s0], f16)
            nc.sync.dma_start(out=xct_sb[:], in_=xct[:, :])
            w1_sb = constp.tile([P, dhid], f16)
            nc.sync.dma_start(out=w1_sb[:], in_=w1[:, :])
            w2_sb = constp.tile([P, dhid], f16)
            nc.sync.dma_start(out=w2_sb[:], in_=w2[:, :])
            w3_sb = constp.tile([P, P], f16)
            nc.sync.dma_start(out=w3_sb[:], in_=w3p[:, :])
            sc0_sb = constp.tile([P, tpc], f32)
            nc.sync.dma_start(out=sc0_sb[:], in_=sc0[:, :])
            sc12_sb = constp.tile([P, tpc], f32)
            nc.sync.dma_start(out=sc12_sb[:], in_=sc12[:, :])
            sc3_sb = constp.tile([P, tpc], f32)
            nc.sync.dma_start(out=sc3_sb[:], in_=sc3[:, :])
            idx_sb = constp.tile([P, idx_cols], i16)
            nc.sync.dma_start(out=idx_sb[:], in_=idx[:, :])
            sel_sb = constp.tile([P, Gtot, P], f8)
            ident = constp.tile([P, P], f16)
            make_identity(nc, ident[:])

            # pre-zero the gather bufs once so unwritten tail slots read 0.0
            for _ in range(GB_BUFS):
                gz = gatherp.tile([P, Gsup, P], f16, tag="gb")
                nc.vector.memset(gz[:], 0.0)

            def chunk_collective(zbuf, li, ch):
                zlo, zhi = cs[ch] * P, (cs[ch] + chunks[ch]) * P
                nc.gpsimd.collective_compute(
                    "AllGather", Alu.bypass, replica_groups=rg,
                    ins=[zbuf[zlo:zhi, :].opt()],
                    outs=[pads[li][ch][:, :].opt()],
                )

            # ---- phase 0: z1 = ns * (X @ W1) -> chunked AllGather -> pads[0]
            # the 14.5MB sel table load is split into per-tile pieces
            # interleaved with the z1 writes so it doesn't head-block the
            # Sync queue (z writes feed the chunk collectives)
            sel_cut = [0] * (tpc + 1)
            for j in range(tpc):
                sel_cut[j + 1] = Gtot * (j + 1) // tpc
            for j in range(tpc):
                zp = psumzp.tile([P, dhid], f32, tag="zp")
                nc.tensor.matmul(
                    out=zp[:], lhsT=xct_sb[:, j * P:(j + 1) * P], rhs=w1_sb[:],
                    start=True, stop=True,
                )
                zt = workp.tile([P, dhid], f16, tag="zt")
                nc.scalar.activation(
                    out=zt[:], in_=zp[:], func=Act.Copy,
                    scale=sc0_sb[:, j:j + 1],
                )
                nc.sync.dma_start(out=z1[j * P:(j + 1) * P, :], in_=zt[:])
                if j in fire_pos:
                    chunk_collective(z1, 0, fire_pos[j])
                g0, g1 = sel_cut[j], sel_cut[j + 1]
                if g1 > g0:
                    nc.sync.dma_start(
                        out=sel_sb[:, g0:g1, :],
                        in_=selt[:, g0 * P:g1 * P],
                    )

            def spmm_layer(li, d_el, scale_sb, wnext_sb, zout, li_out):
                for si, tl in enumerate(sts):
                    gb = gatherp.tile([P, Gsup, P], f16, tag="gb")
                    for (ch, o_g, coff, num) in calls_by_st[si]:
                        K = -(-num // P)
                        nc.gpsimd.dma_gather(
                            out_ap=gb[:, o_g:o_g + K, :],
                            in_ap=pads[li][ch][:, :],
                            idxs_ap=idx_sb[:, coff:coff + (-(-num // 16))],
                            num_idxs=num, num_idxs_reg=num,
                            elem_size=d_el, single_packet=SINGLE_PACKET,
                            queue_num=next_queue(num),
                        )
                    og = og_st[si]
                    for j in tl:
                        glist = []
                        for ch in range(NCH):
                            o, ng = g_of[(j, ch)]
                            glist.extend(range(o, o + ng))
                        acc = psumap.tile([P, P], f32, tag="acc")
                        for gi, g in enumerate(glist):
                            nc.tensor.matmul(
                                out=acc[:], lhsT=sel_sb[:, og + g, :],
                                rhs=gb[:, g, :],
                                start=(gi == 0), stop=(gi == len(glist) - 1),
                            )
                        if wnext_sb is None:
                            ot = workp.tile([P, P], f32, tag="ot")
                            nc.scalar.activation(
                                out=ot[:], in_=acc[:], func=Act.Copy,
                                scale=scale_sb[:, j:j + 1],
                            )
                            nc.sync.dma_start(
                                out=outp[j * P:(j + 1) * P, :], in_=ot[:])
                        else:
                            ht = workp.tile([P, P], f16, tag="ht")
                            nc.scalar.activation(
                                out=ht[:], in_=acc[:], func=Act.Relu,
                                scale=scale_sb[:, j:j + 1],
                            )
                            tp = psumtp.tile([P, P], f16, tag="tp")
                            nc.tensor.transpose(out=tp[:], in_=ht[:],
                                                identity=ident[:])
                            htT = workp.tile([P, P], f16, tag="htT")
                            nc.scalar.activation(out=htT[:], in_=tp[:],
                                                 func=Act.Copy)
                            zp = psumzp.tile([P, P], f32, tag="zp2")
                            nc.tensor.matmul(
                                out=zp[:], lhsT=htT[:], rhs=wnext_sb[:],
                                start=True, stop=True,
                            )
                            zt = workp.tile([P, P], f16, tag="zt2")
                            nc.scalar.activation(out=zt[:], in_=zp[:],
                                                 func=Act.Copy)
                            nc.sync.dma_start(
                                out=zout[j * P:(j + 1) * P, :], in_=zt[:])
                            if j in fire_pos:
                                chunk_collective(zout, li_out, fire_pos[j])

            spmm_layer(0, dhid, sc12_sb, w2_sb, z2, 1)
            spmm_layer(1, dhid, sc12_sb, w3_sb, z3, 2)
            spmm_layer(2, P, sc3_sb, None, None, None)

    nc.compile()
    return nc


# ----------------------------------------------------------------------------
# Entry point
# ----------------------------------------------------------------------------

_CACHE = {}


def _graph_key(edge_index, shapes):
    e = np.asarray(edge_index)
    return (e.shape, hash(e.tobytes()), shapes)


def run(inputs, trace=False, trace_cores=None):
    features = np.asarray(inputs["features"], dtype=np.float32)
    edge_index = np.asarray(inputs["edge_index"])
    W1, b1 = np.asarray(inputs["W1"]), np.asarray(inputs["b1"])
    W2, b2 = np.asarray(inputs["W2"]), np.asarray(inputs["b2"])
    W3, b3 = np.asarray(inputs["W3"]), np.asarray(inputs["b3"])

    meta, in_maps = _preprocess(features, edge_index, W1, b1, W2, b2, W3, b3)
    key = _graph_key(edge_index, (features.shape,))
    if key not in _CACHE:
        _CACHE[key] = _build_program(meta)
    nc = _CACHE[key]

    import concourse.bass_utils as bass_utils

    res = bass_utils.run_bass_kernel_spmd(
        nc, in_maps, core_ids=list(range(NCORES)),
        trace=trace, trace_cores=trace_cores,
    )
    return _assemble(meta, [r["outp"] for r in res.results]), res


def kernel(**inputs):
    return run(inputs)[0]


def _assemble(meta, outs):
    n, ncls, s0 = meta["n"], meta["ncls"], meta["s0"]
    old_of_new = meta["old_of_new"]
    b3 = meta["b3"]
    result = np.empty((n, ncls), dtype=np.float32)
    for c in range(NCORES):
        ids = old_of_new[c * s0:(c + 1) * s0]
        m = ids >= 0
        arr = np.asarray(outs[c])                  # [s0, P]
        result[ids[m]] = arr[m][:, :ncls]
    result += b3[None, :]
    return result
